# Initial kernel scaffold
#
"""Trainium2 Bass kernel for nn_KANLayer (piecewise-constant KAN forward).

Math: reference computes out[b,t,i] = sum_j sum_k basis[b,t,j,k] * coeffs[i,j,k]
where basis k is 1 iff t[max(0,k-2)] <= x < t[min(k+1,9)] with t = linspace(0,1,10).
For x in segment s (t[s] <= x < t[s+1], s in 0..8) exactly k in {s, s+1, s+2} fire, so
    out[t,i] = sum_j D[i,j,seg(x_tj)],  D[i,j,s] = c_s + c_{s+1} + c_{s+2}.
Telescoping over s (step_s = [seg >= s], s=1..8):
    out[t,i] = base[i] + sum_{s=1..8} step_s[t,:] @ G_s[:,i]
with G_s = c_{s+2} - c_{s-1} and base[i] = sum_j D[i,j,0].
This is a K=512*8=4096 matmul with an exact 0/1 bf16 left operand — the minimal
contraction size for this op (9-valued selection -> rank 8 + constant).

Sharding: data-parallel over the 8*2048=16384 tokens, 2048 per NeuronCore;
G/base replicated. Per core: DVE builds 0/1 step planes from host-computed
(bit-exact vs reference) segment indices; PE accumulates 32 K-chunk matmuls
[128x128]x[128x512] bf16 per 128-token tile into PSUM; DVE adds base during
PSUM evacuation.

Perf notes (from ntff traces): MMs pipeline at 216ns (bf16 roofline); the
K-chunk order c = jc*8 + (s-1) + chunk-major accumulation over 4 PSUM banks
keeps early PE demand to the first seg/g slices while the (partition-major,
large-descriptor) DMAs stream in on both HWDGE rings; warmup matmuls on a
scratch tile un-throttle the PE clock (HAM) before real work arrives.
"""

from contextlib import ExitStack

import numpy as np
import ml_dtypes

import concourse.bass as bass  # noqa: F401
import concourse.tile as tile
from concourse import bacc, mybir
from concourse.bass_utils import run_bass_kernel_spmd

N_CORES = 8
TOK_PER_CORE = 2048
IN_F = 512
OUT_F = 512
N_STEP = 8          # step planes s=1..8
JC = IN_F // 128    # 4 j-chunks of 128
KC = N_STEP * JC    # 32 K-chunks of 128
N_GROUPS = 4        # token groups per core
GTOK = TOK_PER_CORE // N_GROUPS   # 512 tokens per group
TT_PER_G = GTOK // 128            # 4 token tiles per group
BF16 = mybir.dt.bfloat16
F32 = mybir.dt.float32

_PROGRAM_CACHE = {}


def _build_program():
    nc = bacc.Bacc("TRN2", target_bir_lowering=False, debug=False)

    # Partition-major layouts: one DMA moves a long contiguous per-partition run.
    seg_d = nc.dram_tensor("seg", [128, JC, TOK_PER_CORE], BF16, kind="ExternalInput").ap()
    g_d = nc.dram_tensor("g", [128, KC, OUT_F], BF16, kind="ExternalInput").ap()
    base_d = nc.dram_tensor("base", [128, OUT_F], F32, kind="ExternalInput").ap()
    out_d = nc.dram_tensor(
        "out", [TOK_PER_CORE // 128, 128, OUT_F], F32, kind="ExternalOutput"
    ).ap()

    with tile.TileContext(nc) as tc, ExitStack() as ctx:
        seg_pool = ctx.enter_context(tc.tile_pool(name="seg", bufs=1))
        g_pool = ctx.enter_context(tc.tile_pool(name="g", bufs=1))
        base_pool = ctx.enter_context(tc.tile_pool(name="base", bufs=1))
        wm_pool = ctx.enter_context(tc.tile_pool(name="wm", bufs=1))
        step_pool = ctx.enter_context(tc.tile_pool(name="step", bufs=2))
        out_pool = ctx.enter_context(tc.tile_pool(name="out", bufs=4))
        # PSUM budget is 8 banks: group-0 accumulates chunk-major across 4
        # banks (bufs=1), later groups go token-tile-major on 1-bank tiles
        # (bufs=4 so evacuation overlaps the next tile's accumulation).
        psum_pool = ctx.enter_context(tc.tile_pool(name="psum", bufs=1, space="PSUM"))
        psum1_pool = ctx.enter_context(tc.tile_pool(name="psum1", bufs=4, space="PSUM"))

        # --- PE warmup: matmuls on a zeroed scratch tile, no DMA dependency.
        # Keeps the PE HAM activity window busy from t=0 so the clock is at
        # 2.4 GHz when the real matmuls start (~3.4us warm-up budget).
        wm = wm_pool.tile([128, 384], BF16, name="wm")
        nc.vector.memset(wm[:], 0.0)
        ps_w = psum_pool.tile([128, TT_PER_G, 512], F32, name="ps")
        for _ in range(20):
            nc.tensor.matmul(
                ps_w[:, 0, :256], wm[:, :128], wm[:, 128:384],
                start=True, stop=True, skip_group_check=True,
            )

        # --- inputs: seg pieces on the sync HWDGE ring, g groups on the
        # scalar HWDGE ring (two independent FIFOs -> parallel streams).
        # seg is split per (group, j-chunk) so the first 128KB piece lands
        # ~1us in and the first step planes can build immediately.
        seg_ts = [
            seg_pool.tile([128, TOK_PER_CORE], BF16, name=f"seg{jc}")
            for jc in range(JC)
        ]
        for q in range(N_GROUPS):
            for jc in range(JC):
                sl = slice(q * GTOK, (q + 1) * GTOK)
                nc.sync.dma_start(seg_ts[jc][:, sl], seg_d[:, jc, sl])

        g_t = g_pool.tile([128, KC, OUT_F], BF16, name="g")
        GSTEP = 4  # chunks per DMA: [128, 4*512] bf16 = 4KB/partition runs
        for gg in range(KC // GSTEP):
            nc.scalar.dma_start(
                g_t[:, gg * GSTEP : (gg + 1) * GSTEP, :],
                g_d[:, gg * GSTEP : (gg + 1) * GSTEP, :],
            )

        base_t = base_pool.tile([128, OUT_F], F32, name="base")
        nc.gpsimd.dma_start(base_t[:], base_d[:])

        # --- main loop: for each 512-token group, build the 32 step planes
        # (chunk c = jc*8 + (s-1): j-chunk-major so early chunks only need
        # seg[0]), then accumulate chunk-major across 4 PSUM banks.
        for q in range(N_GROUPS):
            step_t = step_pool.tile([128, KC, GTOK], BF16, name="step")
            for c in range(KC):
                jc, s = divmod(c, N_STEP)
                nc.vector.tensor_scalar(
                    step_t[:, c, :],
                    seg_ts[jc][:, q * GTOK : (q + 1) * GTOK],
                    float(s + 1) - 0.5,
                    None,
                    mybir.AluOpType.is_ge,
                )
            if q == 0:
                # chunk-major: early matmuls only need the first g/seg slices,
                # so the PE can start while the input DMAs are still streaming.
                ps = psum_pool.tile([128, TT_PER_G, 512], F32, name="ps")
                for c in range(KC):
                    for tt in range(TT_PER_G):
                        nc.tensor.matmul(
                            ps[:, tt, :],
                            step_t[:, c, tt * 128 : (tt + 1) * 128],
                            g_t[:, c, :],
                            start=(c == 0),
                            stop=(c == KC - 1),
                        )
                for tt in range(TT_PER_G):
                    ot = out_pool.tile([128, OUT_F], F32, name="ot")
                    nc.vector.tensor_add(ot[:], ps[:, tt, :], base_t[:])
                    eng = nc.sync if tt % 2 == 0 else nc.scalar
                    eng.dma_start(out_d[q * TT_PER_G + tt], ot[:])
            else:
                # token-tile-major: each tile's accumulation finishes early so
                # its evacuation + output DMA overlap the next tile's matmuls
                # (keeps the post-last-matmul tail to a single tile).
                for tt in range(TT_PER_G):
                    ps1 = psum1_pool.tile([128, 512], F32, name="ps1")
                    for c in range(KC):
                        nc.tensor.matmul(
                            ps1[:],
                            step_t[:, c, tt * 128 : (tt + 1) * 128],
                            g_t[:, c, :],
                            start=(c == 0),
                            stop=(c == KC - 1),
                        )
                    ot = out_pool.tile([128, OUT_F], F32, name="ot")
                    nc.vector.tensor_add(ot[:], ps1[:], base_t[:])
                    eng = nc.sync if tt % 2 == 0 else nc.scalar
                    eng.dma_start(out_d[q * TT_PER_G + tt], ot[:])

    nc.compile()
    return nc


def _get_program():
    if "nc" not in _PROGRAM_CACHE:
        _PROGRAM_CACHE["nc"] = _build_program()
    return _PROGRAM_CACHE["nc"]


def kernel(x: np.ndarray, coeffs: np.ndarray) -> np.ndarray:
    assert x.shape == (8, 2048, IN_F) and coeffs.shape == (OUT_F, IN_F, 12)
    t = np.linspace(0.0, 1.0, 10, dtype=np.float32)  # exact same knots as reference

    # Segment index per element via the same float32 comparisons the
    # reference uses (bit-exact segment assignment).
    xf = np.ascontiguousarray(x.reshape(-1, IN_F))  # [16384, 512]
    seg = np.zeros(xf.shape, dtype=np.float32)
    for m in range(1, 9):
        seg += (xf >= t[m]).astype(np.float32)
    segT = seg.T  # [512 j, 16384 tok]

    c = coeffs.astype(np.float32)
    # G[s-1][j, i] = c[i,j,s+2] - c[i,j,s-1]
    G = np.empty((N_STEP, IN_F, OUT_F), dtype=np.float32)
    for s in range(1, N_STEP + 1):
        G[s - 1] = (c[:, :, s + 2] - c[:, :, s - 1]).T
    # device layout g[p, c, i] with chunk c = jc*8 + (s-1), row p = j - jc*128
    g_dev = np.ascontiguousarray(
        G.reshape(N_STEP, JC, 128, OUT_F).transpose(2, 1, 0, 3).reshape(128, KC, OUT_F)
    ).astype(ml_dtypes.bfloat16)

    base = (c[:, :, 0] + c[:, :, 1] + c[:, :, 2]).sum(axis=1).astype(np.float32)
    base_tile = np.ascontiguousarray(np.broadcast_to(base, (128, OUT_F)))

    # device layout seg[p, jc, t] with row p = j - jc*128
    segT_dev = np.ascontiguousarray(
        segT.reshape(JC, 128, N_CORES * TOK_PER_CORE).transpose(1, 0, 2)
    ).astype(ml_dtypes.bfloat16)

    in_maps = []
    for core in range(N_CORES):
        sl = slice(core * TOK_PER_CORE, (core + 1) * TOK_PER_CORE)
        in_maps.append(
            {
                "seg": np.ascontiguousarray(segT_dev[:, :, sl]),
                "g": g_dev,
                "base": base_tile,
            }
        )

    nc = _get_program()
    res = run_bass_kernel_spmd(nc, in_maps, core_ids=list(range(N_CORES)))
    out = np.stack(
        [res.results[core]["out"].reshape(TOK_PER_CORE, OUT_F) for core in range(N_CORES)]
    )
    return out.astype(np.float32)



# revision 1
# speedup vs baseline: 1.6559x; 1.6559x over previous
"""Trainium2 Bass kernel for nn_KANLayer (piecewise-constant KAN forward).

Math: reference computes out[b,t,i] = sum_j sum_k basis[b,t,j,k] * coeffs[i,j,k]
where basis k is 1 iff t[max(0,k-2)] <= x < t[min(k+1,9)] with t = linspace(0,1,10).
For x in segment s (t[s] <= x < t[s+1], s in 0..8) exactly k in {s, s+1, s+2} fire, so
    out[t,i] = sum_j D[i,j,seg(x_tj)],  D[i,j,s] = c_s + c_{s+1} + c_{s+2}.
Telescoping over s (step_s = [seg >= s], s=1..8):
    out[t,i] = base[i] + sum_{s=1..8} step_s[t,:] @ G_s[:,i]
with G_s = c_{s+2} - c_{s-1} and base[i] = sum_j D[i,j,0].
This is a K=512*8=4096 matmul with an exact 0/1 bf16 left operand — the minimal
contraction size for this op (9-valued selection -> rank 8 + constant).

Sharding: data-parallel over the 8*2048=16384 tokens, 2048 per NeuronCore;
G/base replicated. Per core: DVE builds 0/1 step planes from host-computed
(bit-exact vs reference) segment indices; PE accumulates 32 K-chunk matmuls
[128x128]x[128x512] bf16 per 128-token tile into PSUM; DVE adds base during
PSUM evacuation.

Perf notes (from ntff traces): MMs pipeline at 216ns (bf16 roofline); the
K-chunk order c = jc*8 + (s-1) + chunk-major accumulation over 4 PSUM banks
keeps early PE demand to the first seg/g slices while the (partition-major,
large-descriptor) DMAs stream in on both HWDGE rings; warmup matmuls on a
scratch tile un-throttle the PE clock (HAM) before real work arrives.
"""

from contextlib import ExitStack

import numpy as np
import ml_dtypes

import concourse.bass as bass  # noqa: F401
import concourse.tile as tile
from concourse import bacc, mybir
from concourse.bass_utils import run_bass_kernel_spmd

N_CORES = 8
TOK_PER_CORE = 2048
IN_F = 512
OUT_F = 512
N_STEP = 8          # step planes s=1..8
JC = IN_F // 128    # 4 j-chunks of 128
KC = N_STEP * JC    # 32 K-chunks of 128
N_GROUPS = 4        # token groups per core
GTOK = TOK_PER_CORE // N_GROUPS   # 512 tokens per group
TT_PER_G = GTOK // 128            # 4 token tiles per group
BF16 = mybir.dt.bfloat16
F32 = mybir.dt.float32

_PROGRAM_CACHE = {}


def _build_program():
    nc = bacc.Bacc("TRN2", target_bir_lowering=False, debug=False)

    # Partition-major layouts: one DMA moves a long contiguous per-partition run.
    seg_d = nc.dram_tensor("seg", [128, JC, TOK_PER_CORE], BF16, kind="ExternalInput").ap()
    g_d = nc.dram_tensor("g", [128, KC, OUT_F], BF16, kind="ExternalInput").ap()
    base_d = nc.dram_tensor("base", [128, OUT_F], F32, kind="ExternalInput").ap()
    out_d = nc.dram_tensor(
        "out", [TOK_PER_CORE // 128, 128, OUT_F], F32, kind="ExternalOutput"
    ).ap()

    with tile.TileContext(nc) as tc, ExitStack() as ctx:
        seg_pool = ctx.enter_context(tc.tile_pool(name="seg", bufs=1))
        g_pool = ctx.enter_context(tc.tile_pool(name="g", bufs=1))
        base_pool = ctx.enter_context(tc.tile_pool(name="base", bufs=1))
        wm_pool = ctx.enter_context(tc.tile_pool(name="wm", bufs=1))
        step_pool = ctx.enter_context(tc.tile_pool(name="step", bufs=2))
        out_pool = ctx.enter_context(tc.tile_pool(name="out", bufs=4))
        # PSUM budget is 8 banks: group-0 accumulates chunk-major across 4
        # banks (bufs=1), later groups go token-tile-major on 1-bank tiles
        # (bufs=4 so evacuation overlaps the next tile's accumulation).
        psum_pool = ctx.enter_context(tc.tile_pool(name="psum", bufs=1, space="PSUM"))
        psum1_pool = ctx.enter_context(tc.tile_pool(name="psum1", bufs=4, space="PSUM"))

        # --- PE warmup: matmuls on a zeroed scratch tile, no DMA dependency.
        # Keeps the PE HAM activity window busy from t=0 so the clock is at
        # 2.4 GHz when the real matmuls start (~3.4us warm-up budget).
        wm = wm_pool.tile([128, 384], BF16, name="wm")
        nc.vector.memset(wm[:], 0.0)
        ps_w = psum_pool.tile([128, TT_PER_G, 512], F32, name="ps")
        for _ in range(20):
            nc.tensor.matmul(
                ps_w[:, 0, :256], wm[:, :128], wm[:, 128:384],
                start=True, stop=True, skip_group_check=True,
            )

        # --- inputs: seg pieces on the sync HWDGE ring, g groups on the
        # scalar HWDGE ring (two independent FIFOs -> parallel streams).
        # seg is split per (group, j-chunk) so the first 128KB piece lands
        # ~1us in and the first step planes can build immediately.
        seg_ts = [
            seg_pool.tile([128, TOK_PER_CORE], BF16, name=f"seg{jc}")
            for jc in range(JC)
        ]
        for q in range(N_GROUPS):
            for jc in range(JC):
                sl = slice(q * GTOK, (q + 1) * GTOK)
                nc.sync.dma_start(seg_ts[jc][:, sl], seg_d[:, jc, sl])

        g_t = g_pool.tile([128, KC, OUT_F], BF16, name="g")
        GSTEP = 4  # chunks per DMA: [128, 4*512] bf16 = 4KB/partition runs
        for gg in range(KC // GSTEP):
            nc.scalar.dma_start(
                g_t[:, gg * GSTEP : (gg + 1) * GSTEP, :],
                g_d[:, gg * GSTEP : (gg + 1) * GSTEP, :],
            )

        base_t = base_pool.tile([128, OUT_F], F32, name="base")
        nc.gpsimd.dma_start(base_t[:], base_d[:])

        # --- main loop: for each 512-token group, build the 32 step planes
        # (chunk c = jc*8 + (s-1): j-chunk-major so early chunks only need
        # seg[0]), then accumulate chunk-major across 4 PSUM banks.
        for q in range(N_GROUPS):
            step_t = step_pool.tile([128, KC, GTOK], BF16, name="step")
            for c in range(KC):
                jc, s = divmod(c, N_STEP)
                nc.vector.tensor_scalar(
                    step_t[:, c, :],
                    seg_ts[jc][:, q * GTOK : (q + 1) * GTOK],
                    float(s + 1) - 0.5,
                    None,
                    mybir.AluOpType.is_ge,
                )
            if q == 0:
                # chunk-major: early matmuls only need the first g/seg slices,
                # so the PE can start while the input DMAs are still streaming.
                ps = psum_pool.tile([128, TT_PER_G, 512], F32, name="ps")
                for c in range(KC):
                    for tt in range(TT_PER_G):
                        nc.tensor.matmul(
                            ps[:, tt, :],
                            step_t[:, c, tt * 128 : (tt + 1) * 128],
                            g_t[:, c, :],
                            start=(c == 0),
                            stop=(c == KC - 1),
                        )
                for tt in range(TT_PER_G):
                    ot = out_pool.tile([128, OUT_F], F32, name="ot")
                    nc.vector.tensor_add(ot[:], ps[:, tt, :], base_t[:])
                    eng = nc.sync if tt % 2 == 0 else nc.scalar
                    eng.dma_start(out_d[q * TT_PER_G + tt], ot[:])
            else:
                # token-tile-major: each tile's accumulation finishes early so
                # its evacuation + output DMA overlap the next tile's matmuls
                # (keeps the post-last-matmul tail to a single tile).
                for tt in range(TT_PER_G):
                    ps1 = psum1_pool.tile([128, 512], F32, name="ps1")
                    for c in range(KC):
                        nc.tensor.matmul(
                            ps1[:],
                            step_t[:, c, tt * 128 : (tt + 1) * 128],
                            g_t[:, c, :],
                            start=(c == 0),
                            stop=(c == KC - 1),
                        )
                    ot = out_pool.tile([128, OUT_F], F32, name="ot")
                    nc.vector.tensor_add(ot[:], ps1[:], base_t[:])
                    eng = nc.sync if tt % 2 == 0 else nc.scalar
                    eng.dma_start(out_d[q * TT_PER_G + tt], ot[:])

    nc.compile()
    return nc


def _get_program():
    if "nc" not in _PROGRAM_CACHE:
        _PROGRAM_CACHE["nc"] = _build_program()
    return _PROGRAM_CACHE["nc"]


def kernel(x: np.ndarray, coeffs: np.ndarray) -> np.ndarray:
    assert x.shape == (8, 2048, IN_F) and coeffs.shape == (OUT_F, IN_F, 12)
    t = np.linspace(0.0, 1.0, 10, dtype=np.float32)  # exact same knots as reference

    # Segment index per element via the same float32 comparisons the
    # reference uses (bit-exact segment assignment).
    xf = np.ascontiguousarray(x.reshape(-1, IN_F))  # [16384, 512]
    seg = np.zeros(xf.shape, dtype=np.float32)
    for m in range(1, 9):
        seg += (xf >= t[m]).astype(np.float32)
    segT = seg.T  # [512 j, 16384 tok]

    c = coeffs.astype(np.float32)
    # G[s-1][j, i] = c[i,j,s+2] - c[i,j,s-1]
    G = np.empty((N_STEP, IN_F, OUT_F), dtype=np.float32)
    for s in range(1, N_STEP + 1):
        G[s - 1] = (c[:, :, s + 2] - c[:, :, s - 1]).T
    # device layout g[p, c, i] with chunk c = jc*8 + (s-1), row p = j - jc*128
    g_dev = np.ascontiguousarray(
        G.reshape(N_STEP, JC, 128, OUT_F).transpose(2, 1, 0, 3).reshape(128, KC, OUT_F)
    ).astype(ml_dtypes.bfloat16)

    base = (c[:, :, 0] + c[:, :, 1] + c[:, :, 2]).sum(axis=1).astype(np.float32)
    base_tile = np.ascontiguousarray(np.broadcast_to(base, (128, OUT_F)))

    # device layout seg[p, jc, t] with row p = j - jc*128
    segT_dev = np.ascontiguousarray(
        segT.reshape(JC, 128, N_CORES * TOK_PER_CORE).transpose(1, 0, 2)
    ).astype(ml_dtypes.bfloat16)

    in_maps = []
    for core in range(N_CORES):
        sl = slice(core * TOK_PER_CORE, (core + 1) * TOK_PER_CORE)
        in_maps.append(
            {
                "seg": np.ascontiguousarray(segT_dev[:, :, sl]),
                "g": g_dev,
                "base": base_tile,
            }
        )

    nc = _get_program()
    res = run_bass_kernel_spmd(nc, in_maps, core_ids=list(range(N_CORES)))
    out = np.stack(
        [res.results[core]["out"].reshape(TOK_PER_CORE, OUT_F) for core in range(N_CORES)]
    )
    return out.astype(np.float32)

